# revision 3
# baseline (speedup 1.0000x reference)
"""Trainium2 Bass kernel for the Balance-Theory GNN (nn_Balance_Theory_1580547968962).

Reference computation (all f32):
    x_p = relu(features @ w_p0) @ w_p1            # [N, H]
    x_n = relu(features @ w_n0) @ w_n1            # [N, H]
    feat_p = w_p0*x_p + w_p1*(A_p x_p) + w_p2*(A_p^2 x_p) + w_p3*(A_n^2 x_p)
    feat_n = w_n0*(A_n x_n) + w_n1*(A_p A_n x_n) + w_n2*(A_n A_p x_n)
    z = [feat_p | feat_n]                         # [N, 2H]
    logits = z @ W_prob + bias
    -> (z, log_softmax(logits), argmax(logits), softmax(logits))

Strategy (8 NeuronCores, 1D row/node partition):
  * The 7 adjacency matmuls batch into 2 "levels", so each 256MB adjacency
    matrix is streamed exactly twice (the dependency minimum):
      level 1: A_p @ [x_p | x_n],            A_n @ [x_p | x_n]
      level 2: A_p @ [A_p x_p | A_n x_n],    A_n @ [A_n x_p | A_p x_n]
  * Each core owns 1024 output rows. The host feeds A[rows].T (shape
    [8192, 1024]) so on-device the hop features act as the PE-stationary
    operand (lhsT) and the adjacency tiles stream as the moving operand:
      psum[128 feat, node] += X[k:k+128, :128].T @ A_rows.T[k:k+128, :]
    accumulated over all 64 k-chunks -> (A_local @ X).T with NO on-device
    transpose of the big matrix and 1 weight load per 4 matmuls.
  * Between levels, an AllGather of the [8192, 128] hop features.
  * MLP / weighted sums / softmax head are tiny and run per-core on the
    local 1024 rows.

Memory roofline per core: 2 passes x (32+32)MB adjacency ~= 128MB @
~360GB/s ~= 360us; everything else (~15MB DMA + collectives) rides on top.
"""

import sys

if "/opt/trn_rl_repo" not in sys.path:
    sys.path.insert(0, "/opt/trn_rl_repo")

import numpy as np

NC_CORES = 8
_PROG_CACHE = {}


def _build_program(n_nodes, wp_vals, wn_vals):
    """Emit + compile the per-core SPMD Bass program (identical on all cores)."""
    from contextlib import ExitStack

    import concourse.mybir as mybir
    import concourse.tile as tile
    from concourse import bacc

    f32 = mybir.dt.float32
    nloc = n_nodes // NC_CORES  # rows owned by this core
    kc = n_nodes // 128         # contraction chunks
    mt = nloc // 128            # row tiles owned by this core
    H, C, NF = 64, 10, 256
    assert nloc % 128 == 0 and n_nodes % 128 == 0

    nc = bacc.Bacc(
        "TRN2",
        target_bir_lowering=False,
        debug=False,
        enable_asserts=True,
        num_devices=NC_CORES,
    )

    atp = nc.dram_tensor("atp", [n_nodes, nloc], f32, kind="ExternalInput").ap()
    atn = nc.dram_tensor("atn", [n_nodes, nloc], f32, kind="ExternalInput").ap()
    ftl = nc.dram_tensor("ftl", [NF, nloc], f32, kind="ExternalInput").ap()
    wp0 = nc.dram_tensor("wp0", [NF, H], f32, kind="ExternalInput").ap()
    wp1 = nc.dram_tensor("wp1", [H, H], f32, kind="ExternalInput").ap()
    wn0 = nc.dram_tensor("wn0", [NF, H], f32, kind="ExternalInput").ap()
    wn1 = nc.dram_tensor("wn1", [H, H], f32, kind="ExternalInput").ap()
    wprob = nc.dram_tensor("wprob", [2 * H, C], f32, kind="ExternalInput").ap()
    biasbc = nc.dram_tensor("biasbc", [128, C], f32, kind="ExternalInput").ap()
    ident = nc.dram_tensor("ident", [128, 128], f32, kind="ExternalInput").ap()

    z_out = nc.dram_tensor("z_out", [nloc, 2 * H], f32, kind="ExternalOutput").ap()
    ls_out = nc.dram_tensor("ls_out", [nloc, C], f32, kind="ExternalOutput").ap()
    prob_out = nc.dram_tensor("prob_out", [nloc, C], f32, kind="ExternalOutput").ap()
    pred_out = nc.dram_tensor(
        "pred_out", [nloc, 1], mybir.dt.int32, kind="ExternalOutput"
    ).ap()

    Relu = mybir.ActivationFunctionType.Relu
    Exp = mybir.ActivationFunctionType.Exp
    Ln = mybir.ActivationFunctionType.Ln
    add = mybir.AluOpType.add
    sub = mybir.AluOpType.subtract
    mult = mybir.AluOpType.mult
    maxop = mybir.AluOpType.max
    bypass = mybir.AluOpType.bypass
    AX = mybir.AxisListType.X
    rg = [list(range(NC_CORES))]

    with ExitStack() as ctx:
        tc = ctx.enter_context(tile.TileContext(nc))
        const = ctx.enter_context(tc.tile_pool(name="const", bufs=1))
        sb = ctx.enter_context(tc.tile_pool(name="sb", bufs=1))
        apool = ctx.enter_context(tc.tile_pool(name="apool", bufs=8))
        bpool = ctx.enter_context(tc.tile_pool(name="bpool", bufs=8))
        lpool = ctx.enter_context(tc.tile_pool(name="lpool", bufs=4))
        stg = ctx.enter_context(tc.tile_pool(name="stg", bufs=2))
        pacc = ctx.enter_context(tc.tile_pool(name="pacc", bufs=2, space="PSUM"))
        psm = ctx.enter_context(tc.tile_pool(name="psm", bufs=3, space="PSUM"))
        dram = ctx.enter_context(tc.tile_pool(name="dram", bufs=1, space="DRAM"))

        # ---- constants -------------------------------------------------
        identsb = const.tile([128, 128], f32, tag="identsb")
        nc.sync.dma_start(identsb, ident)
        wp0sb = const.tile([128, 128], f32, tag="wp0sb")
        nc.sync.dma_start(wp0sb[:, 0:64], wp0[0:128, :])
        nc.sync.dma_start(wp0sb[:, 64:128], wp0[128:256, :])
        wn0sb = const.tile([128, 128], f32, tag="wn0sb")
        nc.sync.dma_start(wn0sb[:, 0:64], wn0[0:128, :])
        nc.sync.dma_start(wn0sb[:, 64:128], wn0[128:256, :])
        wp1sb = const.tile([64, 64], f32, tag="wp1sb")
        nc.sync.dma_start(wp1sb, wp1)
        wn1sb = const.tile([64, 64], f32, tag="wn1sb")
        nc.sync.dma_start(wn1sb, wn1)
        wprobsb = const.tile([128, C], f32, tag="wprobsb")
        nc.sync.dma_start(wprobsb, wprob)
        biassb = const.tile([128, C], f32, tag="biassb")
        nc.sync.dma_start(biassb, biasbc)
        ft0 = const.tile([128, nloc], f32, tag="ft0")
        nc.sync.dma_start(ft0, ftl[0:128, :])
        ft1 = const.tile([128, nloc], f32, tag="ft1")
        nc.sync.dma_start(ft1, ftl[128:256, :])

        # ---- MLPs, computed transposed: xT rows 0:64 = x_p.T, 64:128 = x_n.T
        xT = sb.tile([128, nloc], f32, tag="xT")
        for row, w0sb, w1sb in ((0, wp0sb, wp1sb), (64, wn0sb, wn1sb)):
            for n0 in range(0, nloc, 512):
                ss = min(512, nloc - n0)
                ps1 = psm.tile([64, 512], f32, tag="sp")
                nc.tensor.matmul(
                    ps1[:, 0:ss], w0sb[:, 0:64], ft0[:, n0 : n0 + ss],
                    start=True, stop=False,
                )
                nc.tensor.matmul(
                    ps1[:, 0:ss], w0sb[:, 64:128], ft1[:, n0 : n0 + ss],
                    start=False, stop=True,
                )
                t1 = stg.tile([64, 512], f32, tag="t1")
                nc.scalar.activation(t1[:, 0:ss], ps1[:, 0:ss], Relu)
                ps2 = psm.tile([64, 512], f32, tag="sp")
                nc.tensor.matmul(ps2[:, 0:ss], w1sb, t1[:, 0:ss], start=True, stop=True)
                nc.vector.tensor_copy(xT[row : row + 64, n0 : n0 + ss], ps2[:, 0:ss])

        # ---- transpose local x to node-major, AllGather to X full ------
        xg_in = dram.tile([nloc, 128], f32, tag="xg_in")
        xg_out = dram.tile([n_nodes, 128], f32, tag="xg_out", addr_space="Shared")
        for m in range(mt):
            pt = psm.tile([128, 128], f32, tag="sp")
            nc.tensor.transpose(pt, xT[:, m * 128 : (m + 1) * 128], identsb)
            xs = stg.tile([128, 128], f32, tag="xs")
            nc.vector.tensor_copy(xs, pt)
            nc.sync.dma_start(xg_in[m * 128 : (m + 1) * 128, :], xs)
        nc.gpsimd.collective_compute(
            "AllGather", bypass, replica_groups=rg,
            ins=[xg_in.opt()], outs=[xg_out.opt()],
        )

        # ---- level 1: (A_local @ X).T accumulated over 64 k-chunks -----
        p1p = pacc.tile([128, nloc], f32, tag="acc")
        p1n = pacc.tile([128, nloc], f32, tag="acc")
        for c in range(kc):
            csl = slice(c * 128, (c + 1) * 128)
            xc = lpool.tile([128, 128], f32, tag="lh")
            nc.sync.dma_start(xc, xg_out[csl, :])
            at = apool.tile([128, nloc], f32, tag="at")
            nc.sync.dma_start(at, atp[csl, :])
            bt = bpool.tile([128, nloc], f32, tag="bt")
            nc.sync.dma_start(bt, atn[csl, :])
            st, sp_ = (c == 0), (c == kc - 1)
            for s0 in range(0, nloc, 512):
                ss = min(512, nloc - s0)
                ssl = slice(s0, s0 + ss)
                nc.tensor.matmul(p1p[:, ssl], xc, at[:, ssl], start=st, stop=sp_)
                nc.tensor.matmul(p1n[:, ssl], xc, bt[:, ssl], start=st, stop=sp_)
        y1p = sb.tile([128, nloc], f32, tag="y1p")
        nc.vector.tensor_copy(y1p, p1p)
        y1n = sb.tile([128, nloc], f32, tag="y1n")
        nc.vector.tensor_copy(y1n, p1n)

        # ---- transpose level-1 rows to [Ap_xp | An_xn | An_xp | Ap_xn], AG
        yg_in = dram.tile([nloc, 256], f32, tag="yg_in")
        yg_out = dram.tile([n_nodes, 256], f32, tag="yg_out", addr_space="Shared")
        for m in range(mt):
            msl = slice(m * 128, (m + 1) * 128)
            ys = stg.tile([128, 256], f32, tag="ys")
            for src, srow, dcol in (
                (y1p, 0, 0),      # Ap_xp
                (y1n, 64, 64),    # An_xn
                (y1n, 0, 128),    # An_xp
                (y1p, 64, 192),   # Ap_xn
            ):
                pt2 = psm.tile([128, 64], f32, tag="sp")
                # identity block must share the source's base partition
                nc.tensor.transpose(
                    pt2,
                    src[srow : srow + 64, msl],
                    identsb[srow : srow + 64, srow : srow + 64],
                )
                nc.vector.tensor_copy(ys[:, dcol : dcol + 64], pt2)
            nc.sync.dma_start(yg_in[msl, :], ys)
        nc.gpsimd.collective_compute(
            "AllGather", bypass, replica_groups=rg,
            ins=[yg_in.opt()], outs=[yg_out.opt()],
        )

        # ---- level 2 ---------------------------------------------------
        p2p = pacc.tile([128, nloc], f32, tag="acc")
        p2n = pacc.tile([128, nloc], f32, tag="acc")
        for c in range(kc):
            csl = slice(c * 128, (c + 1) * 128)
            gp = lpool.tile([128, 128], f32, tag="lh")
            nc.sync.dma_start(gp, yg_out[csl, 0:128])
            gn = lpool.tile([128, 128], f32, tag="lh2")
            nc.sync.dma_start(gn, yg_out[csl, 128:256])
            at = apool.tile([128, nloc], f32, tag="at")
            nc.sync.dma_start(at, atp[csl, :])
            bt = bpool.tile([128, nloc], f32, tag="bt")
            nc.sync.dma_start(bt, atn[csl, :])
            st, sp_ = (c == 0), (c == kc - 1)
            for s0 in range(0, nloc, 512):
                ss = min(512, nloc - s0)
                ssl = slice(s0, s0 + ss)
                nc.tensor.matmul(p2p[:, ssl], gp, at[:, ssl], start=st, stop=sp_)
                nc.tensor.matmul(p2n[:, ssl], gn, bt[:, ssl], start=st, stop=sp_)
        y2p = sb.tile([128, nloc], f32, tag="y2p")
        nc.vector.tensor_copy(y2p, p2p)
        y2n = sb.tile([128, nloc], f32, tag="y2n")
        nc.vector.tensor_copy(y2n, p2n)

        # ---- hop-weighted sums (still transposed) ----------------------
        # feat_p.T = wp0*x_p.T + wp1*Ap_xp.T + wp2*Ap2_xp.T + wp3*An2_xp.T
        # feat_n.T = wn0*An_xn.T + wn1*Ap_An_xn.T + wn2*An_Ap_xn.T
        # NB: HW birverifier requires both SBUF inputs of scalar_tensor_tensor
        # to share a base partition -- temps are [128, nloc] and each chain
        # stays at its own partition offset (0:64 for feat_p, 64:128 for feat_n).
        zT = sb.tile([128, nloc], f32, tag="zT")
        e1 = stg.tile([128, nloc], f32, tag="ep")
        e2 = stg.tile([128, nloc], f32, tag="ep")
        nc.vector.tensor_scalar_mul(e1[0:64, :], y1p[0:64, :], float(wp_vals[1]))
        nc.vector.scalar_tensor_tensor(
            e2[0:64, :], xT[0:64, :], float(wp_vals[0]), e1[0:64, :],
            op0=mult, op1=add,
        )
        nc.vector.scalar_tensor_tensor(
            e1[0:64, :], y2p[0:64, :], float(wp_vals[2]), e2[0:64, :],
            op0=mult, op1=add,
        )
        nc.vector.scalar_tensor_tensor(
            zT[0:64, :], y2n[0:64, :], float(wp_vals[3]), e1[0:64, :],
            op0=mult, op1=add,
        )
        nc.vector.tensor_scalar_mul(e1[64:128, :], y1n[64:128, :], float(wn_vals[0]))
        nc.vector.scalar_tensor_tensor(
            e2[64:128, :], y2p[64:128, :], float(wn_vals[1]), e1[64:128, :],
            op0=mult, op1=add,
        )
        nc.vector.scalar_tensor_tensor(
            zT[64:128, :], y2n[64:128, :], float(wn_vals[2]), e2[64:128, :],
            op0=mult, op1=add,
        )

        # ---- head: z out, logits, log_softmax, softmax, argmax ---------
        for m in range(mt):
            msl = slice(m * 128, (m + 1) * 128)
            pz = psm.tile([128, 128], f32, tag="sp")
            nc.tensor.transpose(pz, zT[:, msl], identsb)
            zs = stg.tile([128, 128], f32, tag="zs")
            nc.vector.tensor_copy(zs, pz)
            nc.sync.dma_start(z_out[msl, :], zs)

            pl = psm.tile([128, C], f32, tag="sp")
            nc.tensor.matmul(pl, zT[:, msl], wprobsb, start=True, stop=True)
            lg = stg.tile([128, C], f32, tag="lg")
            nc.vector.tensor_add(lg, pl, biassb)

            mx = stg.tile([128, 1], f32, tag="mx")
            nc.vector.tensor_reduce(mx, lg, axis=AX, op=maxop, negate=True)
            ex = stg.tile([128, C], f32, tag="ex")
            se = stg.tile([128, 1], f32, tag="se")
            nc.scalar.activation(
                ex, lg, Exp, bias=mx[:, 0:1], scale=1.0, accum_out=se[:, 0:1]
            )
            lse = stg.tile([128, 1], f32, tag="lse")
            nc.scalar.activation(lse, se, Ln)
            lsm = stg.tile([128, C], f32, tag="lsm")
            nc.vector.tensor_scalar(lsm, lg, mx[:, 0:1], lse[:, 0:1], op0=add, op1=sub)
            nc.sync.dma_start(ls_out[msl, :], lsm)

            rc = stg.tile([128, 1], f32, tag="rc")
            nc.vector.reciprocal(rc, se)
            pr = stg.tile([128, C], f32, tag="pr")
            nc.vector.tensor_scalar_mul(pr, ex, rc[:, 0:1])
            nc.sync.dma_start(prob_out[msl, :], pr)

            m8 = stg.tile([128, 8], f32, tag="m8")
            i8 = stg.tile([128, 8], mybir.dt.uint32, tag="i8")
            nc.vector.max(m8, lg)
            nc.vector.max_index(i8, m8, lg)
            pred = stg.tile([128, 1], mybir.dt.int32, tag="pd")
            nc.vector.tensor_copy(pred, i8[:, 0:1])
            nc.sync.dma_start(pred_out[msl, :], pred)

    nc.compile()
    return nc


def get_program(n_nodes, wp_vals, wn_vals):
    key = (n_nodes, tuple(wp_vals), tuple(wn_vals))
    if key not in _PROG_CACHE:
        _PROG_CACHE[key] = _build_program(n_nodes, wp_vals, wn_vals)
    return _PROG_CACHE[key]


def make_in_maps(inputs):
    """Shard full inputs into per-core in_maps (host-side layout prep)."""
    A_p = np.ascontiguousarray(np.asarray(inputs["A_p"], dtype=np.float32))
    A_n = np.ascontiguousarray(np.asarray(inputs["A_n"], dtype=np.float32))
    feats = np.asarray(inputs["features"], dtype=np.float32)
    n = A_p.shape[0]
    nloc = n // NC_CORES
    bias = np.asarray(inputs["bias"], dtype=np.float32).reshape(1, -1)
    common = {
        "wp0": np.ascontiguousarray(np.asarray(inputs["w_p0"], np.float32)),
        "wp1": np.ascontiguousarray(np.asarray(inputs["w_p1"], np.float32)),
        "wn0": np.ascontiguousarray(np.asarray(inputs["w_n0"], np.float32)),
        "wn1": np.ascontiguousarray(np.asarray(inputs["w_n1"], np.float32)),
        "wprob": np.ascontiguousarray(np.asarray(inputs["W_prob"], np.float32)),
        "biasbc": np.ascontiguousarray(np.broadcast_to(bias, (128, bias.shape[1]))),
        "ident": np.eye(128, dtype=np.float32),
    }
    in_maps = []
    for r in range(NC_CORES):
        rows = slice(r * nloc, (r + 1) * nloc)
        in_maps.append(
            {
                "atp": np.ascontiguousarray(A_p[rows, :].T),
                "atn": np.ascontiguousarray(A_n[rows, :].T),
                "ftl": np.ascontiguousarray(feats[rows, :].T),
                **common,
            }
        )
    return in_maps


def assemble_outputs(results):
    z = np.concatenate([results[r]["z_out"] for r in range(NC_CORES)], axis=0)
    ls = np.concatenate([results[r]["ls_out"] for r in range(NC_CORES)], axis=0)
    prob = np.concatenate([results[r]["prob_out"] for r in range(NC_CORES)], axis=0)
    pred = np.concatenate(
        [results[r]["pred_out"] for r in range(NC_CORES)], axis=0
    ).reshape(-1).astype(np.int32)
    return z, ls, pred, prob


def run_sharded(inputs, trace=False, **run_kwargs):
    """Shard -> run on 8 cores -> gather. Returns ((z, ls, pred, prob), res)."""
    from concourse import bass_utils

    n = np.asarray(inputs["A_p"]).shape[0]
    wp_vals = tuple(float(x) for x in np.asarray(inputs["w_p"]).reshape(-1))
    wn_vals = tuple(float(x) for x in np.asarray(inputs["w_n"]).reshape(-1))
    nc = get_program(n, wp_vals, wn_vals)
    in_maps = make_in_maps(inputs)
    res = bass_utils.run_bass_kernel_spmd(
        nc, in_maps, core_ids=list(range(NC_CORES)), trace=trace, **run_kwargs
    )
    return assemble_outputs(res.results), res


def kernel(**inputs):
    (z, ls, pred, prob), _ = run_sharded(inputs, trace=False)
    return z, ls, pred, prob
